# revision 2
# baseline (speedup 1.0000x reference)
"""ConvTranspose2d (16,256,32,32) -> (16,128,66,66), stride 2, 4x4 kernel.

Strategy: data-parallel over batch, 2 images per core on 8 NeuronCores.

Math: y[b,co,2m+p,2n+q] = bias[co]
        + sum_{i,j in {0,1}} sum_ci x[b,ci,m-i,n-j] * w[ci,co,p+2i,q+2j]
for parity class (p,q) in {0,1}^2, m,n in [0,33).

Per image and parity class the output subgrid [128co x 33 x 33] is
computed in 3 row-chunks; each chunk is one PSUM accumulation group of
8 matmuls (2 ci-chunks x 4 taps (i,j)), K=128, M=128, N=R*34, all in
bf16 (inputs are bf16-quantized host-side; rel err ~2e-3, gate 2e-2).
Shifted taps read a zero-padded 34x34 SBUF copy of x through offset
slices.  PSUM->SBUF drain is a DVE tensor_scalar_add fusing the bias
add, the parity de-interleave, and the fp32->bf16 output cast; the
host upcasts y back to fp32.

Pipeline choices (from trace analysis of the fp32r baseline, 57.2us):
- bf16 halves the input DMA phase (4.4MB -> 2.2MB) which previously
  saturated HBM for 14us and starved the PE until 13.5us.
- Input DMAs are few (9) and issued in consumption order so the
  critical first 0.5MB (class-(0,0) weights + a 5-row x prefix) is not
  bandwidth-shared with a dozen later streams.
- Image 0 runs class-major with chunk rows [3,15,15]: the first matmul
  group needs only x rows [0,5).  Image 1 runs band-major with chunks
  [15,15,3] so the final output DMA is just 6 rows (0.1MB) - the tail
  after the last matmul is DMA-receipt-latency dominated.
- Output leaves as 6 per-band HWDGE DMAs spread over the compute span.
- PE warm-up: HAM starts the PE at 1.2GHz and unthrottles after ~3.4us
  of sustained activity; dummy bf16 matmuls burn the input-DMA ramp.
"""

import numpy as np
import ml_dtypes

import concourse.bass as bass
import concourse.bacc as bacc
import concourse.tile as tile
from concourse import mybir
from concourse.bass_utils import run_bass_kernel_spmd

N_CORES = 8
B_PER = 2  # images per core

F32 = mybir.dt.float32
BF16 = mybir.dt.bfloat16

PW = 34            # padded x width (32 + 1 left + 1 right)
XLEN = PW * PW     # 1156 padded x elems per partition
XPAD = 1160        # sbuf/dram x free size (AP slack for the last chunk)

# per-image chunk plans: list of (m0, R) parity-row chunks covering [0,33)
CHUNKS0 = [(0, 3), (3, 15), (18, 15)]    # small chunk first: early start
CHUNKS1 = [(0, 15), (15, 15), (30, 3)]   # small chunk last: small tail DMA
XBANDS0 = [0, 5 * PW, 19 * PW, XPAD]     # x row-bands gating image-0 chunks


def build_nc(debug: bool = False) -> bass.Bass:
    nc = bacc.Bacc("TRN2", target_bir_lowering=False, debug=debug,
                   num_devices=N_CORES)

    # x arrives host-padded + bf16: [img, ci', c, 34*34+tail]
    x_d = nc.declare_dram_parameter("x", [B_PER, 128, 2, XPAD], BF16,
                                    isOutput=False)
    # w layout: [ci', p, q, c, i, j, co]  (class-major taps, bf16)
    w_d = nc.declare_dram_parameter("w", [128, 2, 2, 2, 2, 2, 128], BF16,
                                    isOutput=False)
    b_d = nc.declare_dram_parameter("b", [128, 1], F32, isOutput=False)
    y_d = nc.declare_dram_parameter("y", [B_PER, 128, 66, 66], BF16,
                                    isOutput=True)

    with tile.TileContext(nc) as tc:
        with (
            tc.tile_pool(name="wp", bufs=1) as wpool,
            tc.tile_pool(name="bp", bufs=2) as bpool,
            tc.tile_pool(name="xp", bufs=B_PER) as xpool,
            tc.tile_pool(name="ybp", bufs=6) as bandpool,
            tc.tile_pool(name="ps", bufs=7, space="PSUM") as ppool,
            tc.tile_pool(name="pw", bufs=1, space="PSUM") as warmpool,
        ):
            # PE warm-up burning the input-DMA ramp
            wub = bpool.tile([128, 512], BF16)
            nc.gpsimd.memset(wub[:], 0.0)
            wps = warmpool.tile([128, 512], F32)
            for _ in range(7):
                nc.tensor.matmul(wps[:], wub[:, 0:128], wub[:],
                                 start=True, stop=True)

            wt = wpool.tile([128, 2, 2, 2, 2, 2, 128], BF16)
            xt = [xpool.tile([128, 2, XPAD], BF16, name=f"x{i}")
                  for i in range(B_PER)]
            bt = bpool.tile([128, 1], F32)

            issue_engines = [nc.sync, nc.scalar, nc.gpsimd]
            issue_i = [0]

            def dma_in(out, in_):
                eng = issue_engines[issue_i[0] % 3]
                issue_i[0] += 1
                eng.dma_start(out=out, in_=in_)

            # consumption-ordered input DMAs
            dma_in(wt[:, 0, 0], w_d[:, 0, 0])
            dma_in(xt[0][:, :, XBANDS0[0]:XBANDS0[1]],
                   x_d[0][:, :, XBANDS0[0]:XBANDS0[1]])
            dma_in(xt[0][:, :, XBANDS0[1]:XBANDS0[2]],
                   x_d[0][:, :, XBANDS0[1]:XBANDS0[2]])
            dma_in(wt[:, 0, 1], w_d[:, 0, 1])
            dma_in(xt[0][:, :, XBANDS0[2]:XBANDS0[3]],
                   x_d[0][:, :, XBANDS0[2]:XBANDS0[3]])
            dma_in(wt[:, 1, 0], w_d[:, 1, 0])
            dma_in(wt[:, 1, 1], w_d[:, 1, 1])
            dma_in(xt[1][:], x_d[1])
            nc.gpsimd.dma_start(out=bt[:], in_=b_d[:])

            def emit_group(ps, img, p, q, m0, R):
                nf = R * PW
                k = 0
                for c in range(2):
                    for i in range(2):
                        for j in range(2):
                            off = (m0 - i + 1) * PW + (1 - j)
                            nc.tensor.matmul(
                                ps[:],
                                wt[:, p, q, c, i, j, :],
                                xt[img][:, c, off:off + nf],
                                start=(k == 0),
                                stop=(k == 7),
                            )
                            k += 1

            def drain(ps, out_view):
                nc.vector.tensor_scalar_add(
                    out_view,
                    ps[:].rearrange("p (m n) -> p m n", n=PW)[:, :, 0:33],
                    bt[:],
                )

            out_engines = [nc.sync, nc.scalar]
            out_i = [0]

            def dma_out(out, in_):
                eng = out_engines[out_i[0] % 2]
                out_i[0] += 1
                eng.dma_start(out=out, in_=in_)

            # ---- image 0: class-major; band DMA when class (1,1) drains ----
            bands0 = [bandpool.tile([128, 2 * R, 66], BF16, name=f"y0b{r}")
                      for r, (m0, R) in enumerate(CHUNKS0)]
            for p in range(2):
                for q in range(2):
                    for r, (m0, R) in enumerate(CHUNKS0):
                        ps = ppool.tile([128, R * PW], F32)
                        emit_group(ps, 0, p, q, m0, R)
                        drain(ps, bands0[r][:, p::2, q::2])
                        if p == 1 and q == 1:
                            dma_out(y_d[0][:, 2 * m0:2 * (m0 + R), :],
                                    bands0[r][:])

            # ---- image 1: band-major; band DMA per chunk ----
            for r, (m0, R) in enumerate(CHUNKS1):
                band = bandpool.tile([128, 2 * R, 66], BF16, name=f"y1b{r}")
                for p in range(2):
                    for q in range(2):
                        ps = ppool.tile([128, R * PW], F32)
                        emit_group(ps, 1, p, q, m0, R)
                        drain(ps, band[:, p::2, q::2])
                dma_out(y_d[1][:, 2 * m0:2 * (m0 + R), :], band[:])

    nc.compile()
    return nc


_nc_cache = None


def _get_nc():
    global _nc_cache
    if _nc_cache is None:
        _nc_cache = build_nc()
    return _nc_cache


def make_in_maps(x: np.ndarray, weight: np.ndarray, bias: np.ndarray):
    bf16 = ml_dtypes.bfloat16
    # w[ci,co,kh,kw] -> [ci', p, q, c, i, j, co]  (kh = 2i+p, kw = 2j+q)
    w7 = (
        weight.astype(np.float32, copy=False)
        .reshape(2, 128, 128, 2, 2, 2, 2)      # [c, ci', co, i, p, j, q]
        .transpose(1, 4, 6, 0, 3, 5, 2)        # -> [ci', p, q, c, i, j, co]
    )
    w_host = np.ascontiguousarray(w7.astype(bf16))
    b_host = np.ascontiguousarray(
        bias.astype(np.float32, copy=False).reshape(128, 1)
    )
    x = np.asarray(x, dtype=np.float32)
    # host-side zero-pad into the 34x34(+tail) layout the kernel reads
    xpad = np.zeros((16, 256, XPAD), dtype=np.float32)
    xpad[:, :, :XLEN].reshape(16, 256, PW, PW)[:, :, 1:33, 1:33] = x
    x_host = np.ascontiguousarray(
        xpad.reshape(16, 2, 128, XPAD).transpose(0, 2, 1, 3).astype(bf16)
    )
    return [
        {
            "x": x_host[B_PER * i:B_PER * (i + 1)],
            "w": w_host,
            "b": b_host,
        }
        for i in range(N_CORES)
    ]


def kernel(x: np.ndarray, weight: np.ndarray, bias: np.ndarray) -> np.ndarray:
    nc = _get_nc()
    in_maps = make_in_maps(x, weight, bias)
    res = run_bass_kernel_spmd(nc, in_maps, list(range(N_CORES)))
    out = np.concatenate(
        [np.asarray(r["y"]).astype(np.float32) for r in res.results], axis=0
    )
    return np.ascontiguousarray(out)


# revision 5
# speedup vs baseline: 1.1292x; 1.1292x over previous
"""ConvTranspose2d (16,256,32,32) -> (16,128,66,66), stride 2, 4x4 kernel.

Strategy: data-parallel over batch, 2 images per core on 8 NeuronCores.

Math: y[b,co,2m+p,2n+q] = bias[co]
        + sum_{i,j in {0,1}} sum_ci x[b,ci,m-i,n-j] * w[ci,co,p+2i,q+2j]
for parity class (p,q) in {0,1}^2, m,n in [0,33).

Per image and parity class the output subgrid [128co x 33 x 33] is
computed in 3 row-chunks; each chunk is one PSUM accumulation group of
8 matmuls (2 ci-chunks x 4 taps (i,j)), K=128, M=128, N=R*34, in bf16
(inputs bf16-quantized host-side; measured MM issue rate is the full
N/2.4GHz+2.5ns -- LDWEIGHTS fully hidden; rel err ~2.3e-3, gate 2e-2).
Shifted taps read a zero-padded 34x34 SBUF copy of x through offset
slices.  PSUM->SBUF drain is a DVE tensor_scalar_add fusing the bias
add and the parity de-interleave, in fp32: bf16 strided drains measured
2.6x slower (sub-word RMW writes) and their ~95% DVE occupancy stole
SBUF bandwidth from the PE stream (257ns/MM instead of 215ns).

Pipeline (from trace analysis of fp32r-baseline 57.2us / bf16-v1 62.3us):
- bf16 halves the input DMA phase (4.4MB -> 2.2MB) which previously
  saturated HBM for 14us and starved the PE until 13.5us.
- SDMA round-robins packets across all queues with pending work, so
  concurrent transfers land ~fair-share late.  Inputs are split into
  phase A (class-(0,0) weights, bias, image-0 x) issued immediately and
  phase B (remaining weights, image-1 x) issued only after a tiny ACT
  gate-op that depends on phase A's last DMA.
- Image 0 runs class-major with chunk rows [3,15,15]: the first matmul
  group needs only a 5-row x prefix.  Image 1 runs band-major with
  [15,15,3] so the final output DMA is 6 rows (0.4MB) -- the tail
  after the last matmul is DMA-receipt-latency dominated.
- Output leaves as 6 per-band HWDGE (sync/scalar) DMAs spread over the
  compute span.
- PE warm-up: HAM starts the PE at 1.2GHz and unthrottles after ~3.4us
  of sustained activity; 4 dummy bf16 matmuls burn the input-DMA ramp.
"""

import numpy as np
import ml_dtypes

import concourse.bass as bass
import concourse.bacc as bacc
import concourse.tile as tile
from concourse import mybir
from concourse.bass_utils import run_bass_kernel_spmd

N_CORES = 8
B_PER = 2  # images per core

F32 = mybir.dt.float32
BF16 = mybir.dt.bfloat16

PW = 34            # padded x width (32 + 1 left + 1 right)
XLEN = PW * PW     # 1156 padded x elems per partition
XPAD = 1160        # sbuf/dram x free size (AP slack for the last chunk)

# per-image chunk plans: list of (m0, R) parity-row chunks covering [0,33)
CHUNKS0 = [(0, 3), (3, 15), (18, 15)]    # small chunk first: early start
CHUNKS1 = [(0, 15), (15, 15), (30, 3)]   # small chunk last: small tail DMA
XBANDS0 = [0, 5 * PW, 19 * PW, XPAD]     # x row-bands gating image-0 chunks


def build_nc(debug: bool = False) -> bass.Bass:
    nc = bacc.Bacc("TRN2", target_bir_lowering=False, debug=debug,
                   num_devices=N_CORES)

    # x arrives host-padded + bf16: [img, ci', c, 34*34+tail]
    x_d = nc.declare_dram_parameter("x", [B_PER, 128, 2, XPAD], BF16,
                                    isOutput=False)
    # w layout: [ci', p, q, c, i, j, co]  (class-major taps, bf16)
    w_d = nc.declare_dram_parameter("w", [128, 2, 2, 2, 2, 2, 128], BF16,
                                    isOutput=False)
    b_d = nc.declare_dram_parameter("b", [128, 1], F32, isOutput=False)
    y_d = nc.declare_dram_parameter("y", [B_PER, 128, 66, 66], F32,
                                    isOutput=True)

    with tile.TileContext(nc) as tc:
        with (
            tc.tile_pool(name="wp", bufs=1) as wpool,
            tc.tile_pool(name="bp", bufs=1) as bpool,
            tc.tile_pool(name="xp", bufs=B_PER) as xpool,
            tc.tile_pool(name="ybp", bufs=6) as bandpool,
            tc.tile_pool(name="ps", bufs=7, space="PSUM") as ppool,
            tc.tile_pool(name="pw", bufs=1, space="PSUM") as warmpool,
        ):
            # PE warm-up burning the input-DMA ramp
            wub = bpool.tile([128, 512], BF16)
            nc.vector.memset(wub[:], 0.0)
            wps = warmpool.tile([128, 512], F32)
            for _ in range(4):
                nc.tensor.matmul(wps[:], wub[:, 0:128], wub[:],
                                 start=True, stop=True)

            wt = wpool.tile([128, 2, 2, 2, 2, 2, 128], BF16)
            xt = [xpool.tile([128, 2, XPAD], BF16, name=f"x{i}", tag="xt")
                  for i in range(B_PER)]
            bt = bpool.tile([128, 1], F32)

            # ---- phase A input DMAs: what image 0's first class needs ----
            nc.sync.dma_start(out=wt[:, 0, 0], in_=w_d[:, 0, 0])
            nc.scalar.dma_start(out=xt[0][:, :, XBANDS0[0]:XBANDS0[1]],
                                in_=x_d[0][:, :, XBANDS0[0]:XBANDS0[1]])
            nc.gpsimd.dma_start(out=bt[:], in_=b_d[:])
            nc.gpsimd.dma_start(out=xt[0][:, :, XBANDS0[1]:XBANDS0[2]],
                                in_=x_d[0][:, :, XBANDS0[1]:XBANDS0[2]])
            nc.gpsimd.dma_start(out=xt[0][:, :, XBANDS0[2]:XBANDS0[3]],
                                in_=x_d[0][:, :, XBANDS0[2]:XBANDS0[3]])

            # ---- phase B, gated on phase A's last DMA (x0 band 3) so the
            # bulk transfers don't steal packet-round-robin bandwidth from
            # the transfers that gate the first matmul groups ----
            gate = bpool.tile([128, 8], BF16)
            nc.scalar.activation(gate[:], xt[0][:, 1, XPAD - 8:XPAD],
                                 mybir.ActivationFunctionType.Copy)
            nc.scalar.dma_start(out=wt[:, 0, 1], in_=w_d[:, 0, 1])
            nc.scalar.dma_start(out=wt[:, 1, 0], in_=w_d[:, 1, 0])
            nc.scalar.dma_start(out=wt[:, 1, 1], in_=w_d[:, 1, 1])
            nc.scalar.dma_start(out=xt[1][:], in_=x_d[1])

            def emit_group(ps, img, p, q, m0, R):
                nf = R * PW
                k = 0
                for c in range(2):
                    for i in range(2):
                        for j in range(2):
                            off = (m0 - i + 1) * PW + (1 - j)
                            nc.tensor.matmul(
                                ps[:],
                                wt[:, p, q, c, i, j, :],
                                xt[img][:, c, off:off + nf],
                                start=(k == 0),
                                stop=(k == 7),
                            )
                            k += 1

            def drain(ps, out_view):
                nc.vector.tensor_scalar_add(
                    out_view,
                    ps[:].rearrange("p (m n) -> p m n", n=PW)[:, :, 0:33],
                    bt[:],
                )

            out_engines = [nc.sync, nc.scalar]
            out_i = [0]

            def dma_out(out, in_):
                eng = out_engines[out_i[0] % 2]
                out_i[0] += 1
                eng.dma_start(out=out, in_=in_)

            # ---- image 0: class-major; band DMA when class (1,1) drains ----
            bands0 = [bandpool.tile([128, 30, 66], F32, name=f"y0b{r}", tag="yb")
                      for r in range(3)]
            for p in range(2):
                for q in range(2):
                    for r, (m0, R) in enumerate(CHUNKS0):
                        ps = ppool.tile([128, R * PW], F32)
                        emit_group(ps, 0, p, q, m0, R)
                        drain(ps, bands0[r][:, 0:2 * R][:, p::2, q::2])
                        if p == 1 and q == 1:
                            dma_out(y_d[0][:, 2 * m0:2 * (m0 + R), :],
                                    bands0[r][:, 0:2 * R])

            # ---- image 1: band-major; band DMA per chunk ----
            for r, (m0, R) in enumerate(CHUNKS1):
                band = bandpool.tile([128, 30, 66], F32, name=f"y1b{r}", tag="yb")
                for p in range(2):
                    for q in range(2):
                        ps = ppool.tile([128, R * PW], F32)
                        emit_group(ps, 1, p, q, m0, R)
                        drain(ps, band[:, 0:2 * R][:, p::2, q::2])
                dma_out(y_d[1][:, 2 * m0:2 * (m0 + R), :], band[:, 0:2 * R])

    nc.compile()
    return nc


_nc_cache = None


def _get_nc():
    global _nc_cache
    if _nc_cache is None:
        _nc_cache = build_nc()
    return _nc_cache


def make_in_maps(x: np.ndarray, weight: np.ndarray, bias: np.ndarray):
    bf16 = ml_dtypes.bfloat16
    # w[ci,co,kh,kw] -> [ci', p, q, c, i, j, co]  (kh = 2i+p, kw = 2j+q)
    w7 = (
        weight.astype(np.float32, copy=False)
        .reshape(2, 128, 128, 2, 2, 2, 2)      # [c, ci', co, i, p, j, q]
        .transpose(1, 4, 6, 0, 3, 5, 2)        # -> [ci', p, q, c, i, j, co]
    )
    w_host = np.ascontiguousarray(w7.astype(bf16))
    b_host = np.ascontiguousarray(
        bias.astype(np.float32, copy=False).reshape(128, 1)
    )
    x = np.asarray(x, dtype=np.float32)
    # host-side zero-pad into the 34x34(+tail) layout the kernel reads
    xpad = np.zeros((16, 256, XPAD), dtype=np.float32)
    xpad[:, :, :XLEN].reshape(16, 256, PW, PW)[:, :, 1:33, 1:33] = x
    x_host = np.ascontiguousarray(
        xpad.reshape(16, 2, 128, XPAD).transpose(0, 2, 1, 3).astype(bf16)
    )
    return [
        {
            "x": x_host[B_PER * i:B_PER * (i + 1)],
            "w": w_host,
            "b": b_host,
        }
        for i in range(N_CORES)
    ]


def kernel(x: np.ndarray, weight: np.ndarray, bias: np.ndarray) -> np.ndarray:
    nc = _get_nc()
    in_maps = make_in_maps(x, weight, bias)
    res = run_bass_kernel_spmd(nc, in_maps, list(range(N_CORES)))
    out = np.concatenate([r["y"] for r in res.results], axis=0)
    return np.ascontiguousarray(out.astype(np.float32, copy=False))


# revision 8
# speedup vs baseline: 1.1796x; 1.0446x over previous
"""ConvTranspose2d (16,256,32,32) -> (16,128,66,66), stride 2, 4x4 kernel.

Strategy: data-parallel over batch, 2 images per core on 8 NeuronCores.

Math: y[b,co,2m+p,2n+q] = bias[co]
        + sum_{i,j in {0,1}} sum_ci x[b,ci,m-i,n-j] * w[ci,co,p+2i,q+2j]
for parity class (p,q) in {0,1}^2, m,n in [0,33).

Per image and parity class the output subgrid [128co x 33 x 33] is
computed in 3 row-chunks; each chunk is one PSUM accumulation group of
8 matmuls (2 ci-chunks x 4 taps (i,j)), K=128, M=128, N=R*34, in bf16
(inputs bf16-quantized host-side; measured MM issue rate is the full
N/2.4GHz+2.5ns -- LDWEIGHTS fully hidden; rel err ~2.3e-3, gate 2e-2).
Shifted taps read a zero-padded 34x34 SBUF copy of x through offset
slices.  PSUM->SBUF drain is a DVE tensor_scalar_add fusing the bias
add and the parity de-interleave, in fp32: bf16 strided drains measured
2.6x slower (sub-word RMW writes) and their ~95% DVE occupancy stole
SBUF bandwidth from the PE stream (257ns/MM instead of 215ns).

Pipeline (from trace analysis of fp32r-baseline 57.2us / bf16-v1 62.3us):
- bf16 halves the input DMA phase (4.4MB -> 2.2MB) which previously
  saturated HBM for 14us and starved the PE until 13.5us.
- SDMA round-robins packets across all queues with pending work, so
  concurrent transfers land ~fair-share late.  Inputs are split into
  phase A (class-(0,0) weights, bias, image-0 x) issued immediately and
  phase B (remaining weights, image-1 x) issued only after a tiny ACT
  gate-op that depends on phase A's last DMA.
- Image 0 runs class-major with chunk rows [3,15,15]: the first matmul
  group needs only a 5-row x prefix.  Image 1 runs band-major with
  [15,15,3] so the final output DMA is 6 rows (0.4MB) -- the tail
  after the last matmul is DMA-receipt-latency dominated.
- Output leaves as 6 per-band HWDGE (sync/scalar) DMAs spread over the
  compute span.
- PE warm-up: HAM starts the PE at 1.2GHz and unthrottles after ~3.4us
  of sustained activity; 4 dummy bf16 matmuls burn the input-DMA ramp.
"""

import numpy as np
import ml_dtypes

import concourse.bass as bass
import concourse.bacc as bacc
import concourse.tile as tile
from concourse import mybir
from concourse.bass_utils import run_bass_kernel_spmd

N_CORES = 8
B_PER = 2  # images per core

F32 = mybir.dt.float32
BF16 = mybir.dt.bfloat16

PW = 34            # padded x width (32 + 1 left + 1 right)
XLEN = PW * PW     # 1156 padded x elems per partition
XPAD = 1160        # sbuf/dram x free size (AP slack for the last chunk)

# per-image chunk plans: list of (m0, R) parity-row chunks covering [0,33)
CHUNKS0 = [(0, 3), (3, 15), (18, 15)]    # small chunk first: early start
CHUNKS1 = [(0, 15), (15, 15), (30, 3)]   # small chunk last: small tail DMA
# x row-bands gating image-0 chunks (band ends past each chunk's last read)
XBANDS0 = [0, 5 * PW, 20 * PW, XPAD]


def build_nc(debug: bool = False) -> bass.Bass:
    nc = bacc.Bacc("TRN2", target_bir_lowering=False, debug=debug,
                   num_devices=N_CORES)

    # x arrives host-padded + bf16: [img, ci', c, 34*34+tail]
    x_d = nc.declare_dram_parameter("x", [B_PER, 128, 2, XPAD], BF16,
                                    isOutput=False)
    # w layout: [ci', p, q, c, i, j, co]  (class-major taps, bf16)
    w_d = nc.declare_dram_parameter("w", [128, 2, 2, 2, 2, 2, 128], BF16,
                                    isOutput=False)
    b_d = nc.declare_dram_parameter("b", [128, 1], F32, isOutput=False)
    y_d = nc.declare_dram_parameter("y", [B_PER, 128, 66, 66], F32,
                                    isOutput=True)

    with tile.TileContext(nc) as tc:
        with (
            tc.tile_pool(name="wp", bufs=1) as wpool,
            tc.tile_pool(name="bp", bufs=1) as bpool,
            tc.tile_pool(name="xp", bufs=B_PER) as xpool,
            tc.tile_pool(name="ybp", bufs=6) as bandpool,
            tc.tile_pool(name="ps", bufs=7, space="PSUM") as ppool,
            tc.tile_pool(name="pw", bufs=1, space="PSUM") as warmpool,
        ):
            # PE warm-up burning the input-DMA ramp
            wub = bpool.tile([128, 512], BF16)
            nc.gpsimd.memset(wub[:], 0.0)
            wps = warmpool.tile([128, 512], F32)
            for _ in range(3):
                nc.tensor.matmul(wps[:], wub[:, 0:128], wub[:],
                                 start=True, stop=True)

            wt = wpool.tile([128, 2, 2, 2, 2, 2, 128], BF16)
            xt = [xpool.tile([128, 2, XPAD], BF16, name=f"x{i}", tag="xt")
                  for i in range(B_PER)]
            bt = bpool.tile([128, 1], F32)

            # ---- input DMAs: per-queue FIFO gives in-order landing, the
            # two HWDGE queues (sync=SP, scalar=ACT) each sustain only
            # ~115-190 GB/s, so every transfer is split by ci-chunk across
            # both queues and issued in consumption order.  Only x0's tiny
            # 5-row prefix (which gates the first matmul group) rides the
            # slower SWDGE path so it doesn't queue behind w00. ----
            def xslice(img, c, lo, hi):
                return xt[img][:, c, lo:hi], x_d[img][:, c, lo:hi]

            a0, a1 = XBANDS0[0], XBANDS0[1]
            o, i_ = xslice(0, 0, a0, a1)
            nc.gpsimd.dma_start(out=o, in_=i_)
            o, i_ = xslice(0, 1, a0, a1)
            nc.gpsimd.dma_start(out=o, in_=i_)

            nc.sync.dma_start(out=bt[:], in_=b_d[:])
            nc.sync.dma_start(out=wt[:, 0, 0, 0], in_=w_d[:, 0, 0, 0])
            nc.scalar.dma_start(out=wt[:, 0, 0, 1], in_=w_d[:, 0, 0, 1])
            for b0, b1 in ((XBANDS0[1], XBANDS0[2]),
                           (XBANDS0[2], XBANDS0[3])):
                o, i_ = xslice(0, 0, b0, b1)
                nc.sync.dma_start(out=o, in_=i_)
                o, i_ = xslice(0, 1, b0, b1)
                nc.scalar.dma_start(out=o, in_=i_)
            for p, q in ((0, 1), (1, 0), (1, 1)):
                nc.sync.dma_start(out=wt[:, p, q, 0], in_=w_d[:, p, q, 0])
                nc.scalar.dma_start(out=wt[:, p, q, 1], in_=w_d[:, p, q, 1])
            o, i_ = xslice(1, 0, 0, XPAD)
            nc.sync.dma_start(out=o, in_=i_)
            o, i_ = xslice(1, 1, 0, XPAD)
            nc.scalar.dma_start(out=o, in_=i_)

            def emit_group(ps, img, p, q, m0, R):
                nf = R * PW
                k = 0
                for c in range(2):
                    for i in range(2):
                        for j in range(2):
                            off = (m0 - i + 1) * PW + (1 - j)
                            nc.tensor.matmul(
                                ps[:],
                                wt[:, p, q, c, i, j, :],
                                xt[img][:, c, off:off + nf],
                                start=(k == 0),
                                stop=(k == 7),
                            )
                            k += 1

            def drain(ps, out_view):
                nc.vector.tensor_scalar_add(
                    out_view,
                    ps[:].rearrange("p (m n) -> p m n", n=PW)[:, :, 0:33],
                    bt[:],
                )

            out_engines = [nc.sync, nc.scalar]
            out_i = [0]

            def dma_out(out, in_):
                eng = out_engines[out_i[0] % 2]
                out_i[0] += 1
                eng.dma_start(out=out, in_=in_)

            # ---- image 0: class-major; band DMA when class (1,1) drains ----
            bands0 = [bandpool.tile([128, 30, 66], F32, name=f"y0b{r}", tag="yb")
                      for r in range(3)]
            for p in range(2):
                for q in range(2):
                    for r, (m0, R) in enumerate(CHUNKS0):
                        ps = ppool.tile([128, R * PW], F32)
                        emit_group(ps, 0, p, q, m0, R)
                        drain(ps, bands0[r][:, 0:2 * R][:, p::2, q::2])
                        if p == 1 and q == 1:
                            dma_out(y_d[0][:, 2 * m0:2 * (m0 + R), :],
                                    bands0[r][:, 0:2 * R])

            # ---- image 1: band-major; band DMA per chunk.  The final band
            # is the post-compute tail: split it across both HWDGE queues
            # (each only sustains ~115-190 GB/s). ----
            for r, (m0, R) in enumerate(CHUNKS1):
                band = bandpool.tile([128, 30, 66], F32, name=f"y1b{r}", tag="yb")
                for p in range(2):
                    for q in range(2):
                        ps = ppool.tile([128, R * PW], F32)
                        emit_group(ps, 1, p, q, m0, R)
                        drain(ps, band[:, 0:2 * R][:, p::2, q::2])
                if r < 2:
                    dma_out(y_d[1][:, 2 * m0:2 * (m0 + R), :],
                            band[:, 0:2 * R])
                else:
                    nc.sync.dma_start(out=y_d[1][:, 2 * m0:2 * m0 + R, :],
                                      in_=band[:, 0:R])
                    nc.scalar.dma_start(
                        out=y_d[1][:, 2 * m0 + R:2 * (m0 + R), :],
                        in_=band[:, R:2 * R])

    nc.compile()
    return nc


_nc_cache = None


def _get_nc():
    global _nc_cache
    if _nc_cache is None:
        _nc_cache = build_nc()
    return _nc_cache


def make_in_maps(x: np.ndarray, weight: np.ndarray, bias: np.ndarray):
    bf16 = ml_dtypes.bfloat16
    # w[ci,co,kh,kw] -> [ci', p, q, c, i, j, co]  (kh = 2i+p, kw = 2j+q)
    w7 = (
        weight.astype(np.float32, copy=False)
        .reshape(2, 128, 128, 2, 2, 2, 2)      # [c, ci', co, i, p, j, q]
        .transpose(1, 4, 6, 0, 3, 5, 2)        # -> [ci', p, q, c, i, j, co]
    )
    w_host = np.ascontiguousarray(w7.astype(bf16))
    b_host = np.ascontiguousarray(
        bias.astype(np.float32, copy=False).reshape(128, 1)
    )
    x = np.asarray(x, dtype=np.float32)
    # host-side zero-pad into the 34x34(+tail) layout the kernel reads
    xpad = np.zeros((16, 256, XPAD), dtype=np.float32)
    xpad[:, :, :XLEN].reshape(16, 256, PW, PW)[:, :, 1:33, 1:33] = x
    x_host = np.ascontiguousarray(
        xpad.reshape(16, 2, 128, XPAD).transpose(0, 2, 1, 3).astype(bf16)
    )
    return [
        {
            "x": x_host[B_PER * i:B_PER * (i + 1)],
            "w": w_host,
            "b": b_host,
        }
        for i in range(N_CORES)
    ]


def kernel(x: np.ndarray, weight: np.ndarray, bias: np.ndarray) -> np.ndarray:
    nc = _get_nc()
    in_maps = make_in_maps(x, weight, bias)
    res = run_bass_kernel_spmd(nc, in_maps, list(range(N_CORES)))
    out = np.concatenate([r["y"] for r in res.results], axis=0)
    return np.ascontiguousarray(out.astype(np.float32, copy=False))
